# revision 1
# baseline (speedup 1.0000x reference)
"""TP-8 decode attention kernel for TRN2 (Bass/Tile).

Shards the 8 KV heads (and their 2 q heads each) across 8 NeuronCores.
Per core: qkv projection (1/8 of columns), RoPE, scores vs its K-cache
shard, softmax with new-token fixup, probs@V, out-proj partial (1/8 of
rows). Host sums the 8 partial outputs (the out_proj all-reduce).

All compute-engine accesses keep partition base 0 (HW quadrant rule):
per-batch score rows are produced by accumulating batch-masked qT
matmuls into one [16, N] PSUM tile; the V product is computed
transposed (V tiles as lhsT) so outputs land on d-partitions.
"""

import sys

sys.path.insert(0, "/opt/trn_rl_repo")

import numpy as np

B, S, C = 8, 1, 4096
DIM = 3072
HQ, HKV, HD = 16, 8, 256
REP = HQ // HKV  # 2
NCORES = 8
SCALE = HD ** (-0.5)


def build_bass():
    import concourse.bass as bass  # noqa: F401
    import concourse.mybir as mybir
    import concourse.tile as tile
    from concourse import bacc
    from contextlib import ExitStack

    f32 = mybir.dt.float32
    Alu = mybir.AluOpType
    Act = mybir.ActivationFunctionType

    nc = bacc.Bacc("TRN2", target_bir_lowering=False, debug=False,
                   num_devices=NCORES)

    xT = nc.dram_tensor("xT", [128, 24 * B], f32, kind="ExternalInput").ap()
    wqkv = nc.dram_tensor("wqkv", [24, 128, 1024], f32, kind="ExternalInput").ap()
    kT = nc.dram_tensor("kT", [B, 2, 128, C], f32, kind="ExternalInput").ap()
    vv = nc.dram_tensor("vv", [B, 8, 128, 1024], f32, kind="ExternalInput").ap()
    wout = nc.dram_tensor("wout", [4, 128, DIM], f32, kind="ExternalInput").ap()
    fm = nc.dram_tensor("fm", [16, C], f32, kind="ExternalInput").ap()
    cs4 = nc.dram_tensor("cs4", [128, 4], f32, kind="ExternalInput").ap()
    ident = nc.dram_tensor("ident", [128, 128], f32, kind="ExternalInput").ap()
    mkv = nc.dram_tensor("mkv", [16, 1], f32, kind="ExternalInput").ap()
    dup = nc.dram_tensor("dup", [B, 16], f32, kind="ExternalInput").ap()
    cmask = nc.dram_tensor("cmask", [128, B * 16], f32, kind="ExternalInput").ap()
    ones8 = nc.dram_tensor("ones8", [1, 128], f32, kind="ExternalInput").ap()
    y = nc.dram_tensor("y", [B, DIM], f32, kind="ExternalOutput").ap()

    with tile.TileContext(nc) as tc, ExitStack() as stk:
        io = stk.enter_context(tc.tile_pool(name="io", bufs=1))
        tmpp = stk.enter_context(tc.tile_pool(name="tmp", bufs=4))
        wp = stk.enter_context(tc.tile_pool(name="wp", bufs=3))
        kvp = stk.enter_context(tc.tile_pool(name="kvp", bufs=20))
        wop = stk.enter_context(tc.tile_pool(name="wop", bufs=3))
        ps = stk.enter_context(tc.tile_pool(name="ps", bufs=8, space="PSUM"))

        # ---- small constants ----
        xT_sb = io.tile([128, 24 * B], f32, tag="xT")
        nc.sync.dma_start(xT_sb[:], xT)
        fm_sb = io.tile([16, C], f32, tag="fm")
        nc.sync.dma_start(fm_sb[:], fm)
        cs_sb = io.tile([128, 4], f32, tag="cs")
        nc.sync.dma_start(cs_sb[:], cs4)
        id_sb = io.tile([128, 128], f32, tag="id")
        nc.sync.dma_start(id_sb[:], ident)
        mkv_sb = io.tile([16, 1], f32, tag="mkv")
        nc.sync.dma_start(mkv_sb[:], mkv)
        dup_sb = io.tile([B, 16], f32, tag="dup")
        nc.sync.dma_start(dup_sb[:], dup)
        cm_sb = io.tile([128, B * 16], f32, tag="cm")
        nc.sync.dma_start(cm_sb[:], cmask)
        on_sb = io.tile([1, 128], f32, tag="on")
        nc.sync.dma_start(on_sb[:], ones8)
        cos_s, sin_s = cs_sb[:, 0:1], cs_sb[:, 1:2]
        cos_p, sin_p = cs_sb[:, 2:3], cs_sb[:, 3:4]

        # ---- phase 1: qkvT = W_shard^T @ x^T  (8 chunks of [128, B]) ----
        chunks = [ps.tile([128, B], f32, tag="ps", name=f"qkvT{i}")
                  for i in range(8)]
        for t in range(24):
            wt = wp.tile([128, 1024], f32, tag="w")
            nc.sync.dma_start(wt[:], wqkv[t])
            for c in range(8):
                nc.tensor.matmul(chunks[c][:], wt[:, c * 128:(c + 1) * 128],
                                 xT_sb[:, t * B:(t + 1) * B],
                                 start=(t == 0), stop=(t == 23))

        # ---- rope ----
        qTh = [io.tile([128, 16], f32, tag=f"qTh{h}", name=f"qTh{h}")
               for h in range(2)]
        knT = [io.tile([128, B], f32, tag=f"knT{h}", name=f"knT{h}")
               for h in range(2)]

        def rope(c1, c2, cosa, sina, out1, out2):
            ta = tmpp.tile([128, B], f32, tag="tmp", name="ta")
            tb = tmpp.tile([128, B], f32, tag="tmp", name="tb")
            nc.vector.tensor_scalar_mul(ta[:], c1, cosa)
            nc.vector.tensor_scalar_mul(tb[:], c2, sina)
            nc.vector.tensor_tensor(out1, ta[:], tb[:], op=Alu.subtract)
            tc_ = tmpp.tile([128, B], f32, tag="tmp", name="tc_")
            td = tmpp.tile([128, B], f32, tag="tmp", name="td")
            nc.vector.tensor_scalar_mul(tc_[:], c1, sina)
            nc.vector.tensor_scalar_mul(td[:], c2, cosa)
            nc.vector.tensor_tensor(out2, tc_[:], td[:], op=Alu.add)

        for r in range(2):
            o1 = qTh[0][:].rearrange("p (b r) -> p r b", r=2)[:, r]
            o2 = qTh[1][:].rearrange("p (b r) -> p r b", r=2)[:, r]
            rope(chunks[2 * r][:], chunks[2 * r + 1][:], cos_s, sin_s, o1, o2)
        rope(chunks[4][:], chunks[5][:], cos_p, sin_p, knT[0][:], knT[1][:])

        # batch-masked qT copies: qThM[b][h] has only cols 2b,2b+1 nonzero
        qThM = [[io.tile([128, 16], f32, tag=f"qM{b}_{h}", name=f"qM{b}_{h}")
                 for h in range(2)] for b in range(B)]
        for b in range(B):
            for h in range(2):
                nc.vector.tensor_tensor(qThM[b][h][:], qTh[h][:],
                                        cm_sb[:, b * 16:(b + 1) * 16],
                                        op=Alu.mult)

        # v_newT chunks -> sbuf [128(d),B] and row-major [B,128] halves
        vnT = [io.tile([128, B], f32, tag=f"vnT{h}", name=f"vnT{h}")
               for h in range(2)]
        vn_row = [io.tile([B, 128], f32, tag=f"vnr{h}", name=f"vnr{h}")
                  for h in range(2)]
        for h in range(2):
            nc.scalar.copy(vnT[h][:], chunks[6 + h][:])
            pvt = ps.tile([B, 128], f32, tag="ps")
            nc.tensor.transpose(pvt[:], vnT[h][:], id_sb[:])
            nc.scalar.copy(vn_row[h][:], pvt[:])

        # ---- s_new[16,1] via masked accumulation (+ mask[kv]) ----
        psn = ps.tile([16, 1], f32, tag="ps")
        for b in range(B):
            for h in range(2):
                nc.tensor.matmul(psn[:], qThM[b][h][:], knT[h][:, b:b + 1],
                                 start=(b == 0 and h == 0),
                                 stop=(b == B - 1 and h == 1))
        s_new = io.tile([16, 1], f32, tag="snew")
        nc.vector.tensor_scalar_add(s_new[:], psn[:], mkv_sb[:, 0:1])

        # ---- phase 2: scores, masked-accumulated over batches ----
        scores = io.tile([16, C], f32, tag="scores")
        mparts = io.tile([16, 8], f32, tag="mparts")
        for g in range(4):  # c-range groups of 1024
            kt = {}
            for b in range(B):
                for h in range(2):
                    kk = kvp.tile([128, 1024], f32, tag="kv",
                                  name=f"k{g}_{b}_{h}")
                    nc.sync.dma_start(kk[:],
                                      kT[b, h][:, g * 1024:(g + 1) * 1024])
                    kt[(b, h)] = kk
            for j in range(2):
                pch = ps.tile([16, 512], f32, tag="ps")
                first = True
                for b in range(B):
                    for h in range(2):
                        nc.tensor.matmul(pch[:], qThM[b][h][:],
                                         kt[(b, h)][:, j * 512:(j + 1) * 512],
                                         start=first,
                                         stop=(b == B - 1 and h == 1))
                        first = False
                ssl = slice(g * 1024 + j * 512, g * 1024 + (j + 1) * 512)
                nc.vector.tensor_tensor(scores[:, ssl], pch[:],
                                        fm_sb[:, ssl], op=Alu.add)
                nc.vector.tensor_reduce(mparts[:, g * 2 + j: g * 2 + j + 1],
                                        scores[:, ssl],
                                        axis=mybir.AxisListType.X, op=Alu.max)

        # ---- softmax (kv col killed by fm; new token via rank-1) ----
        m1 = io.tile([16, 1], f32, tag="m1")
        nc.vector.tensor_reduce(m1[:], mparts[:], axis=mybir.AxisListType.X,
                                op=Alu.max)
        tmax = io.tile([16, 1], f32, tag="tmax")
        nc.vector.tensor_tensor(tmax[:], m1[:], s_new[:], op=Alu.max)
        negmax = io.tile([16, 1], f32, tag="negmax")
        nc.vector.tensor_scalar_mul(negmax[:], tmax[:], -1.0)
        sumz = io.tile([16, 1], f32, tag="sumz")
        nc.scalar.activation(scores[:], scores[:], Act.Exp, bias=negmax[:],
                             accum_out=sumz[:])
        p_kv = io.tile([16, 1], f32, tag="pkv")
        nc.scalar.activation(p_kv[:], s_new[:], Act.Exp, bias=negmax[:])
        norm = io.tile([16, 1], f32, tag="norm")
        nc.vector.tensor_tensor(norm[:], sumz[:], p_kv[:], op=Alu.add)
        rnorm = io.tile([16, 1], f32, tag="rnorm")
        nc.vector.reciprocal(rnorm[:], norm[:])
        # rnB[128,16]: rnorm broadcast down partitions (for end scaling)
        prt = ps.tile([1, 16], f32, tag="ps")
        nc.tensor.transpose(prt[:], rnorm[:], id_sb[:16, :16])
        rnT = io.tile([1, 16], f32, tag="rnT")
        nc.scalar.copy(rnT[:], prt[:])
        prb = ps.tile([128, 16], f32, tag="ps")
        nc.tensor.matmul(prb[:], on_sb[:], rnT[:], start=True, stop=True)
        rnB = io.tile([128, 16], f32, tag="rnB")
        nc.scalar.copy(rnB[:], prb[:])

        # probsT via PE transpose: 32 x [16,128] -> [128,16]
        probsT = io.tile([128, 32 * 16], f32, tag="probsT")
        for ct in range(32):
            pt = ps.tile([128, 16], f32, tag="ps")
            nc.tensor.transpose(pt[:], scores[:, ct * 128:(ct + 1) * 128],
                                id_sb[:16, :16])
            nc.scalar.copy(probsT[:, ct * 16:(ct + 1) * 16], pt[:])

        # selP[b', 2b+r] = delta(b',b) * pkvn[2b+r]
        pnt = ps.tile([1, 16], f32, tag="ps")
        nc.tensor.transpose(pnt[:], p_kv[:], id_sb[:16, :16])
        pkvnT = io.tile([1, 16], f32, tag="pkvnT")
        nc.scalar.copy(pkvnT[:], pnt[:])
        pob = ps.tile([B, 16], f32, tag="ps")
        nc.tensor.matmul(pob[:], on_sb[:, 0:B], pkvnT[:], start=True, stop=True)
        pkvB = io.tile([B, 16], f32, tag="pkvB")
        nc.scalar.copy(pkvB[:], pob[:])
        selP = io.tile([B, 16], f32, tag="selP")
        nc.vector.tensor_tensor(selP[:], dup_sb[:], pkvB[:], op=Alu.mult)

        # ---- phase 3: avT_{b,h2}[128(d),2(r)] = sum_ct V_ct^T @ probsT ----
        aTt = [io.tile([128, B], f32, tag=f"aT{t}", name=f"aT{t}")
               for t in range(4)]
        for b in range(B):
            vts = []
            for q in range(8):
                vtile = kvp.tile([128, 1024], f32, tag="kv", name=f"v{b}_{q}")
                nc.sync.dma_start(vtile[:], vv[b, q])
                vts.append(vtile)
            for h2 in range(2):
                pav = ps.tile([128, 2], f32, tag="ps")
                for q in range(8):
                    for sl in range(4):
                        ct = q * 4 + sl
                        nc.tensor.matmul(
                            pav[:],
                            vts[q][:, sl * 256 + h2 * 128:
                                   sl * 256 + (h2 + 1) * 128],
                            probsT[:, ct * 16 + 2 * b: ct * 16 + 2 * b + 2],
                            start=(ct == 0), stop=False)
                nc.tensor.matmul(pav[:], vn_row[h2][:],
                                 selP[:, 2 * b:2 * b + 2],
                                 start=False, stop=True)
                for r in range(2):
                    nc.vector.tensor_tensor(
                        aTt[r * 2 + h2][:, b:b + 1], pav[:, r:r + 1],
                        rnB[:, 2 * b + r: 2 * b + r + 1], op=Alu.mult)

        # ---- phase 4: y = aT.T @ W_out_shard ----
        y_sb = io.tile([B, DIM], f32, tag="ysb")
        pys = [ps.tile([B, 512], f32, tag="ps", name=f"py{n}")
               for n in range(6)]
        for t in range(4):
            wt2 = wop.tile([128, DIM], f32, tag="wo")
            nc.sync.dma_start(wt2[:], wout[t])
            for nch in range(6):
                nc.tensor.matmul(pys[nch][:], aTt[t][:],
                                 wt2[:, nch * 512:(nch + 1) * 512],
                                 start=(t == 0), stop=(t == 3))
        for nch in range(6):
            nc.scalar.copy(y_sb[:, nch * 512:(nch + 1) * 512], pys[nch][:])
        nc.sync.dma_start(y, y_sb[:])

    nc.compile()
    return nc


_CACHED = {}


def _get_bass():
    if "nc" not in _CACHED:
        _CACHED["nc"] = build_bass()
    return _CACHED["nc"]


def _prep_inputs(x, freqs_cos, freqs_sin, kv, k_cache, v_cache, mask,
                 W_qkv, W_out):
    x2 = np.asarray(x, np.float32).reshape(B, DIM)
    xT192 = np.ascontiguousarray(
        x2.T.reshape(24, 128, B).transpose(1, 0, 2).reshape(128, 24 * B))
    cos = np.asarray(freqs_cos, np.float32)[0]
    sin = np.asarray(freqs_sin, np.float32)[0]
    cs4 = np.ascontiguousarray(
        np.stack([cos * SCALE, sin * SCALE, cos, sin], 1), np.float32)
    kvp = int(np.asarray(kv).reshape(-1)[0])
    maskr = np.asarray(mask, np.float32)
    fm = np.tile(maskr, (16, 1)).astype(np.float32)
    fm[:, kvp] -= 1e30
    mkv = np.full((16, 1), maskr[0, kvp], np.float32)
    ident = np.eye(128, dtype=np.float32)
    dupm = np.zeros((B, 16), np.float32)
    for b in range(B):
        dupm[b, 2 * b] = 1.0
        dupm[b, 2 * b + 1] = 1.0
    cmask = np.zeros((128, B * 16), np.float32)
    for b in range(B):
        cmask[:, b * 16 + 2 * b] = 1.0
        cmask[:, b * 16 + 2 * b + 1] = 1.0
    ones8 = np.ones((1, 128), np.float32)
    kc = np.asarray(k_cache, np.float32)
    vc = np.asarray(v_cache, np.float32)
    Wq = np.asarray(W_qkv, np.float32)
    Wo = np.asarray(W_out, np.float32)

    in_maps = []
    for m in range(NCORES):
        wq_shard = np.concatenate([
            Wq[:, 2 * m * HD:(2 * m + 2) * HD],
            Wq[:, HQ * HD + m * HD: HQ * HD + (m + 1) * HD],
            Wq[:, (HQ + HKV) * HD + m * HD: (HQ + HKV) * HD + (m + 1) * HD],
        ], axis=1)
        wq_shard = np.ascontiguousarray(wq_shard).reshape(24, 128, 1024)
        kTs = np.ascontiguousarray(
            kc[:, :, m, :].transpose(0, 2, 1)).reshape(B, 2, 128, C)
        vsh = np.ascontiguousarray(
            vc[:, :, m, :].reshape(B, 8, 4, 128, HD).transpose(0, 1, 3, 2, 4)
        ).reshape(B, 8, 128, 1024)
        wo_shard = np.ascontiguousarray(
            Wo[m * 2 * HD:(m + 1) * 2 * HD, :]).reshape(4, 128, DIM)
        in_maps.append({
            "xT": xT192, "wqkv": wq_shard, "kT": kTs, "vv": vsh,
            "wout": wo_shard, "fm": fm, "cs4": cs4, "ident": ident,
            "mkv": mkv, "dup": dupm, "cmask": cmask, "ones8": ones8,
        })
    return in_maps


def _run(inputs, trace=False):
    from concourse.bass_utils import run_bass_kernel_spmd
    nc = _get_bass()
    in_maps = _prep_inputs(**inputs)
    res = run_bass_kernel_spmd(nc, in_maps, core_ids=list(range(NCORES)),
                               trace=trace)
    parts = [r["y"] for r in res.results]
    out = np.sum(np.stack(parts, 0), 0, dtype=np.float32)
    return out.reshape(B, S, DIM), res


def kernel(**inputs):
    out, _ = _run(inputs, trace=False)
    return out



# revision 2
# speedup vs baseline: 2.4716x; 2.4716x over previous
"""TP-8 decode attention kernel for TRN2 (Bass/Tile), bf16 streaming.

Shards the 8 KV heads (2 q heads each) across 8 NeuronCores. Per core:
qkv projection (1/8 of columns), RoPE, scores vs its K-cache shard,
softmax with new-token fixup, probs@V, out-proj partial (1/8 of rows).
Host sums the 8 partial outputs (the out_proj all-reduce).

Key perf structure vs the fp32 v1:
- all large operands (x, W_qkv, K, V, W_out, probs) are bf16: halves HBM
  traffic (43MB/core) and removes the fp32 matmul penalty.
- few large DMAs (2-3MB each) instead of 165 x 512KB.
- qkv projection runs x-stationary (weights are the tiny operand, W
  streams as the moving operand): 48 matmuls, trivial LDWEIGHTS.
- probs@V runs per batch with probsT columns as a 2-wide stationary
  operand and V streaming 256-wide: 264 matmuls, trivial LDWEIGHTS.
- scores accumulate into one [16, 512] PSUM chunk via batch-masked q
  tiles (16 matmuls per chunk, rhs = that batch's K slice).

All compute-engine accesses keep partition base 0; partition placement
is done only by matmul/transpose (PE) and DMA.
"""

import sys

sys.path.insert(0, "/opt/trn_rl_repo")

import numpy as np

B, S, C = 8, 1, 4096
DIM = 3072
HQ, HKV, HD = 16, 8, 256
REP = HQ // HKV  # 2
NCORES = 8
SCALE = HD ** (-0.5)


def build_bass():
    import concourse.bass as bass  # noqa: F401
    import concourse.mybir as mybir
    import concourse.tile as tile
    from concourse import bacc
    from contextlib import ExitStack

    f32 = mybir.dt.float32
    bf16 = mybir.dt.bfloat16
    Alu = mybir.AluOpType
    Act = mybir.ActivationFunctionType

    nc = bacc.Bacc("TRN2", target_bir_lowering=False, debug=False,
                   num_devices=NCORES)

    # DRAM inputs (host-prepped layouts; see _prep_inputs)
    xT = nc.dram_tensor("xT", [128, 24 * B], bf16, kind="ExternalInput").ap()
    wq = nc.dram_tensor("wq", [3, 128, 8192], bf16, kind="ExternalInput").ap()
    kt = nc.dram_tensor("kt", [8, 128, 8192], bf16, kind="ExternalInput").ap()
    vt = nc.dram_tensor("vt", [8, 128, 8192], bf16, kind="ExternalInput").ap()
    wo = nc.dram_tensor("wo", [128, 4 * DIM], bf16, kind="ExternalInput").ap()
    fm = nc.dram_tensor("fm", [16, C], f32, kind="ExternalInput").ap()
    cs4 = nc.dram_tensor("cs4", [128, 4], f32, kind="ExternalInput").ap()
    identf = nc.dram_tensor("identf", [128, 128], f32,
                            kind="ExternalInput").ap()
    cmask = nc.dram_tensor("cmask", [128, 128], f32, kind="ExternalInput").ap()
    dup = nc.dram_tensor("dup", [B, 16], f32, kind="ExternalInput").ap()
    ones1 = nc.dram_tensor("ones1", [1, 128], f32, kind="ExternalInput").ap()
    mkv = nc.dram_tensor("mkv", [16, 1], f32, kind="ExternalInput").ap()
    y = nc.dram_tensor("y", [B, DIM], f32, kind="ExternalOutput").ap()

    with tile.TileContext(nc) as tc, ExitStack() as stk:
        io = stk.enter_context(tc.tile_pool(name="io", bufs=1))
        wp = stk.enter_context(tc.tile_pool(name="wp", bufs=2))
        kp = stk.enter_context(tc.tile_pool(name="kp", bufs=2))
        vp = stk.enter_context(tc.tile_pool(name="vp", bufs=2))
        ap_ = stk.enter_context(tc.tile_pool(name="ap", bufs=2))
        ps = stk.enter_context(tc.tile_pool(name="ps", bufs=8, space="PSUM"))

        # ---- small persistent constants ----
        xT_sb = io.tile([128, 24 * B], bf16, tag="xT")
        nc.sync.dma_start(xT_sb[:], xT)
        cs_sb = io.tile([128, 4], f32, tag="cs")
        nc.sync.dma_start(cs_sb[:], cs4)
        id_sb = io.tile([128, 128], f32, tag="id")
        nc.sync.dma_start(id_sb[:], identf)
        cm_sb = io.tile([128, 128], f32, tag="cm")
        nc.sync.dma_start(cm_sb[:], cmask)
        dup_sb = io.tile([B, 16], f32, tag="dup")
        nc.sync.dma_start(dup_sb[:], dup)
        on_sb = io.tile([1, 128], f32, tag="on")
        nc.sync.dma_start(on_sb[:], ones1)
        mkv_sb = io.tile([16, 1], f32, tag="mkv")
        nc.sync.dma_start(mkv_sb[:], mkv)
        fm_sb = io.tile([16, C], f32, tag="fm")
        nc.sync.dma_start(fm_sb[:], fm)
        cos_s, sin_s = cs_sb[:, 0:1], cs_sb[:, 1:2]
        cos_p, sin_p = cs_sb[:, 2:3], cs_sb[:, 3:4]

        # ---- phase 1: qkv rows = x @ Wq_shard; x stationary, W moving ----
        psq = [ps.tile([B, 512], f32, tag="ps", name=f"psq{j}")
               for j in range(2)]
        for ci in range(3):
            wt = wp.tile([128, 8192], bf16, tag="wq")
            nc.sync.dma_start(wt[:], wq[ci])
            for il in range(8):
                t = ci * 8 + il
                lhsT = xT_sb[:, t * B:(t + 1) * B]
                for j2 in range(2):
                    nc.tensor.matmul(psq[j2][:], lhsT,
                                     wt[:, il * 1024 + j2 * 512:
                                        il * 1024 + (j2 + 1) * 512],
                                     start=(t == 0), stop=(t == 23))
        qkv_sb = io.tile([B, 1024], f32, tag="qkv")
        nc.scalar.copy(qkv_sb[:, 0:512], psq[0][:])
        nc.scalar.copy(qkv_sb[:, 512:1024], psq[1][:])
        # v_new rows, straight to bf16
        vn_sb = io.tile([B, 256], bf16, tag="vn")
        nc.scalar.copy(vn_sb[:], psq[1][:, 256:512])

        # ---- phase 2: transposes + rope + batch-masked q tiles ----
        # q slices [8, 128] -> [128, 8] per (h, dh); k slices likewise
        qt_raw = [[io.tile([128, B], f32, tag=f"qr{h}{dh}")
                   for dh in range(2)] for h in range(2)]
        for h in range(2):
            for dh in range(2):
                pt = ps.tile([128, B], f32, tag="ps")
                nc.tensor.transpose(
                    pt[:], qkv_sb[:, h * 256 + dh * 128:
                                  h * 256 + (dh + 1) * 128],
                    id_sb[:B, :B])
                nc.scalar.copy(qt_raw[h][dh][:], pt[:])
        kn_raw = [io.tile([128, B], f32, tag=f"kr{dh}") for dh in range(2)]
        for dh in range(2):
            pt = ps.tile([128, B], f32, tag="ps")
            nc.tensor.transpose(pt[:], qkv_sb[:, 512 + dh * 128:
                                              512 + (dh + 1) * 128],
                                id_sb[:B, :B])
            nc.scalar.copy(kn_raw[dh][:], pt[:])

        def rope(c1, c2, cosa, sina, out1, out2):
            ta = io.tile([128, B], f32, tag="rta", name="rta")
            tb = io.tile([128, B], f32, tag="rtb", name="rtb")
            nc.vector.tensor_scalar_mul(ta[:], c1, cosa)
            nc.vector.tensor_scalar_mul(tb[:], c2, sina)
            nc.vector.tensor_tensor(out1, ta[:], tb[:], op=Alu.subtract)
            nc.vector.tensor_scalar_mul(ta[:], c1, sina)
            nc.vector.tensor_scalar_mul(tb[:], c2, cosa)
            nc.vector.tensor_tensor(out2, ta[:], tb[:], op=Alu.add)

        # qTh[dh] [128, 16] f32, col = 2b + h
        qTh = [io.tile([128, 16], f32, tag=f"qTh{dh}") for dh in range(2)]
        for h in range(2):
            o1 = qTh[0][:].rearrange("p (b r) -> p r b", r=2)[:, h]
            o2 = qTh[1][:].rearrange("p (b r) -> p r b", r=2)[:, h]
            rope(qt_raw[h][0][:], qt_raw[h][1][:], cos_s, sin_s, o1, o2)
        # knT[dh] [128, 8] bf16
        knT = [io.tile([128, B], bf16, tag=f"knT{dh}") for dh in range(2)]
        rope(kn_raw[0][:], kn_raw[1][:], cos_p, sin_p, knT[0][:], knT[1][:])

        # batch-masked q tiles (bf16): only cols 2b, 2b+1 nonzero
        Mt = [[io.tile([128, 16], bf16, tag=f"Mt{b}_{dh}")
               for dh in range(2)] for b in range(B)]
        for b in range(B):
            for dh in range(2):
                nc.vector.tensor_tensor(Mt[b][dh][:], qTh[dh][:],
                                        cm_sb[:, b * 16:(b + 1) * 16],
                                        op=Alu.mult)

        # ---- s_new[16,1] (+ mask[kv]) ----
        psn = ps.tile([16, 1], f32, tag="ps")
        for b in range(B):
            for dh in range(2):
                nc.tensor.matmul(psn[:], Mt[b][dh][:], knT[dh][:, b:b + 1],
                                 start=(b == 0 and dh == 0),
                                 stop=(b == B - 1 and dh == 1))
        s_new = io.tile([16, 1], f32, tag="snew")
        nc.vector.tensor_scalar_add(s_new[:], psn[:], mkv_sb[:, 0:1])

        # ---- phase 3: scores [16, C] via masked accumulation ----
        scores = io.tile([16, C], f32, tag="scores")
        mparts = io.tile([16, 8], f32, tag="mparts")
        for g in range(8):
            ktile = kp.tile([128, 8192], bf16, tag="kt")
            nc.sync.dma_start(ktile[:], kt[g])
            pch = ps.tile([16, 512], f32, tag="ps")
            for b in range(B):
                for dh in range(2):
                    nc.tensor.matmul(pch[:], Mt[b][dh][:],
                                     ktile[:, (b * 2 + dh) * 512:
                                           (b * 2 + dh + 1) * 512],
                                     start=(b == 0 and dh == 0),
                                     stop=(b == B - 1 and dh == 1))
            ssl = slice(g * 512, (g + 1) * 512)
            nc.vector.tensor_tensor(scores[:, ssl], pch[:], fm_sb[:, ssl],
                                    op=Alu.add)
            nc.vector.tensor_reduce(mparts[:, g:g + 1], scores[:, ssl],
                                    axis=mybir.AxisListType.X, op=Alu.max)

        # out-proj weights: fetch between K and V streams
        wo_sb = io.tile([128, 4 * DIM], bf16, tag="wo")
        nc.sync.dma_start(wo_sb[:], wo)

        # ---- phase 4: softmax (new token via rank-1 fixup) ----
        m1 = io.tile([16, 1], f32, tag="m1")
        nc.vector.tensor_reduce(m1[:], mparts[:], axis=mybir.AxisListType.X,
                                op=Alu.max)
        tmax = io.tile([16, 1], f32, tag="tmax")
        nc.vector.tensor_tensor(tmax[:], m1[:], s_new[:], op=Alu.max)
        negmax = io.tile([16, 1], f32, tag="negmax")
        nc.vector.tensor_scalar_mul(negmax[:], tmax[:], -1.0)
        sumz = io.tile([16, 1], f32, tag="sumz")
        nc.scalar.activation(scores[:], scores[:], Act.Exp, bias=negmax[:],
                             accum_out=sumz[:])
        p_kv = io.tile([16, 1], f32, tag="pkv")
        nc.scalar.activation(p_kv[:], s_new[:], Act.Exp, bias=negmax[:])
        norm = io.tile([16, 1], f32, tag="norm")
        nc.vector.tensor_tensor(norm[:], sumz[:], p_kv[:], op=Alu.add)
        rnorm = io.tile([16, 1], f32, tag="rnorm")
        nc.vector.reciprocal(rnorm[:], norm[:])
        # rnB [128, 16]: rnorm broadcast down partitions
        prt = ps.tile([1, 16], f32, tag="ps")
        nc.tensor.transpose(prt[:], rnorm[:], id_sb[:16, :16])
        rnT = io.tile([1, 16], f32, tag="rnT")
        nc.scalar.copy(rnT[:], prt[:])
        prb = ps.tile([128, 16], f32, tag="ps")
        nc.tensor.matmul(prb[:], on_sb[:], rnT[:], start=True, stop=True)
        rnB = io.tile([128, 16], f32, tag="rnB")
        nc.scalar.copy(rnB[:], prb[:])
        # selPn[b', 2b+r] = delta(b',b) * p_new[2b+r] * rnorm[2b+r]  (bf16)
        pnt = ps.tile([1, 16], f32, tag="ps")
        nc.tensor.transpose(pnt[:], p_kv[:], id_sb[:16, :16])
        pkvnT = io.tile([1, 16], f32, tag="pkvnT")
        nc.scalar.copy(pkvnT[:], pnt[:])
        pob = ps.tile([B, 16], f32, tag="ps")
        nc.tensor.matmul(pob[:], on_sb[:, 0:B], pkvnT[:], start=True,
                         stop=True)
        pkvB = io.tile([B, 16], f32, tag="pkvB")
        nc.scalar.copy(pkvB[:], pob[:])
        selP = io.tile([B, 16], f32, tag="selP")
        nc.vector.tensor_tensor(selP[:], dup_sb[:], pkvB[:], op=Alu.mult)
        selPn = io.tile([B, 16], bf16, tag="selPn")
        nc.vector.tensor_tensor(selPn[:], selP[:], rnB[:B, :], op=Alu.mult)

        # ---- phase 5: probsT (normalized, bf16) via PE transpose ----
        probsT = io.tile([128, 32 * 16], bf16, tag="probsT")
        for ct in range(32):
            pt = ps.tile([128, 16], f32, tag="ps")
            nc.tensor.transpose(pt[:], scores[:, ct * 128:(ct + 1) * 128],
                                id_sb[:16, :16])
            nc.vector.tensor_tensor(probsT[:, ct * 16:(ct + 1) * 16], pt[:],
                                    rnB[:], op=Alu.mult)

        # ---- phase 6: attn = probs @ V per batch (M=2), transpose to aT ----
        aT4 = [io.tile([128, B], bf16, tag=f"aT{t}") for t in range(4)]
        for b in range(B):
            vtile = vp.tile([128, 8192], bf16, tag="vt")
            nc.sync.dma_start(vtile[:], vt[b])
            pab = ps.tile([2, 256], f32, tag="ps")
            for ct in range(32):
                nc.tensor.matmul(pab[:],
                                 probsT[:, ct * 16 + 2 * b:
                                        ct * 16 + 2 * b + 2],
                                 vtile[:, ct * 256:(ct + 1) * 256],
                                 start=(ct == 0), stop=False)
            nc.tensor.matmul(pab[:], selPn[:, 2 * b:2 * b + 2], vn_sb[:],
                             start=False, stop=True)
            attn_b = ap_.tile([2, 256], f32, tag="attn")
            nc.scalar.copy(attn_b[:], pab[:])
            for dh in range(2):
                pta = ps.tile([128, 2], f32, tag="ps")
                nc.tensor.transpose(pta[:],
                                    attn_b[:, dh * 128:(dh + 1) * 128],
                                    id_sb[:2, :2])
                for h in range(2):
                    nc.scalar.copy(aT4[h * 2 + dh][:, b:b + 1],
                                   pta[:, h:h + 1])

        # ---- phase 7: y = attn @ Wo_shard ----
        y_sb = io.tile([B, DIM], f32, tag="ysb")
        for n in range(6):
            py = ps.tile([B, 512], f32, tag="ps")
            for t in range(4):
                nc.tensor.matmul(py[:], aT4[t][:],
                                 wo_sb[:, t * DIM + n * 512:
                                       t * DIM + (n + 1) * 512],
                                 start=(t == 0), stop=(t == 3))
            nc.scalar.copy(y_sb[:, n * 512:(n + 1) * 512], py[:])
        nc.sync.dma_start(y, y_sb[:])

    nc.compile()
    return nc


_CACHED = {}


def _get_bass():
    if "nc" not in _CACHED:
        _CACHED["nc"] = build_bass()
    return _CACHED["nc"]


def _prep_inputs(x, freqs_cos, freqs_sin, kv, k_cache, v_cache, mask,
                 W_qkv, W_out):
    import ml_dtypes

    bf = ml_dtypes.bfloat16
    x2 = np.asarray(x, np.float32).reshape(B, DIM)
    xT192 = np.ascontiguousarray(
        x2.T.reshape(24, 128, B).transpose(1, 0, 2).reshape(128, 24 * B)
    ).astype(bf)
    cos = np.asarray(freqs_cos, np.float32)[0]
    sin = np.asarray(freqs_sin, np.float32)[0]
    cs4 = np.ascontiguousarray(
        np.stack([cos * SCALE, sin * SCALE, cos, sin], 1), np.float32)
    kvp = int(np.asarray(kv).reshape(-1)[0])
    maskr = np.asarray(mask, np.float32)
    fm = np.tile(maskr, (16, 1)).astype(np.float32)
    fm[:, kvp] -= 1e30
    mkv = np.full((16, 1), maskr[0, kvp], np.float32)
    identf = np.eye(128, dtype=np.float32)
    dupm = np.zeros((B, 16), np.float32)
    for b in range(B):
        dupm[b, 2 * b] = 1.0
        dupm[b, 2 * b + 1] = 1.0
    cmask = np.zeros((128, 128), np.float32)
    for b in range(B):
        cmask[:, b * 16 + 2 * b] = 1.0
        cmask[:, b * 16 + 2 * b + 1] = 1.0
    ones1 = np.ones((1, 128), np.float32)
    kc = np.asarray(k_cache, np.float32)
    vc = np.asarray(v_cache, np.float32)
    Wq = np.asarray(W_qkv, np.float32)
    Wo = np.asarray(W_out, np.float32)

    in_maps = []
    for m in range(NCORES):
        wq_shard = np.concatenate([
            Wq[:, 2 * m * HD:(2 * m + 2) * HD],
            Wq[:, HQ * HD + m * HD: HQ * HD + (m + 1) * HD],
            Wq[:, (HQ + HKV) * HD + m * HD: (HQ + HKV) * HD + (m + 1) * HD],
        ], axis=1)  # [3072, 1024]
        wq3 = np.ascontiguousarray(
            wq_shard.reshape(3, 8, 128, 1024).transpose(0, 2, 1, 3)
            .reshape(3, 128, 8192)).astype(bf)
        kc_m = kc[:, :, m, :]  # [B, C, 256]
        kt8 = np.ascontiguousarray(
            kc_m.reshape(B, 8, 512, 2, 128).transpose(1, 4, 0, 3, 2)
            .reshape(8, 128, 8192)).astype(bf)
        vc_m = vc[:, :, m, :]  # [B, C, 256]
        vt8 = np.ascontiguousarray(
            vc_m.reshape(B, 32, 128, 256).transpose(0, 2, 1, 3)
            .reshape(B, 128, 8192)).astype(bf)
        wo_shard = np.ascontiguousarray(
            Wo[m * 2 * HD:(m + 1) * 2 * HD, :].reshape(4, 128, DIM)
            .transpose(1, 0, 2).reshape(128, 4 * DIM)).astype(bf)
        in_maps.append({
            "xT": xT192, "wq": wq3, "kt": kt8, "vt": vt8, "wo": wo_shard,
            "fm": fm, "cs4": cs4, "identf": identf, "cmask": cmask,
            "dup": dupm, "ones1": ones1, "mkv": mkv,
        })
    return in_maps


def _run(inputs, trace=False):
    from concourse.bass_utils import run_bass_kernel_spmd
    nc = _get_bass()
    in_maps = _prep_inputs(**inputs)
    res = run_bass_kernel_spmd(nc, in_maps, core_ids=list(range(NCORES)),
                               trace=trace)
    parts = [r["y"] for r in res.results]
    out = np.sum(np.stack(parts, 0), 0, dtype=np.float32)
    return out.reshape(B, S, DIM), res


def kernel(**inputs):
    out, _ = _run(inputs, trace=False)
    return out


# revision 6
# speedup vs baseline: 3.1013x; 1.2548x over previous
"""TP-8 decode attention kernel for TRN2 (Bass/Tile), bf16 streaming.

Shards the 8 KV heads (2 q heads each) across 8 NeuronCores. Per core:
qkv projection (1/8 of columns), RoPE, scores vs its K-cache shard,
softmax with new-token fixup, probs@V, out-proj partial (1/8 of rows).
Host sums the 8 partial outputs (the out_proj all-reduce).

Key perf structure vs the fp32 v1:
- all large operands (x, W_qkv, K, V, W_out, probs) are bf16: halves HBM
  traffic (43MB/core) and removes the fp32 matmul penalty.
- few large DMAs (2-3MB each) instead of 165 x 512KB.
- qkv projection runs x-stationary (weights are the tiny operand, W
  streams as the moving operand): 48 matmuls, trivial LDWEIGHTS.
- probs@V runs per batch with probsT columns as a 2-wide stationary
  operand and V streaming 256-wide: 264 matmuls, trivial LDWEIGHTS.
- scores accumulate into one [16, 512] PSUM chunk via batch-masked q
  tiles (16 matmuls per chunk, rhs = that batch's K slice).

All compute-engine accesses keep partition base 0; partition placement
is done only by matmul/transpose (PE) and DMA.
"""

import sys

sys.path.insert(0, "/opt/trn_rl_repo")

import numpy as np

B, S, C = 8, 1, 4096
DIM = 3072
HQ, HKV, HD = 16, 8, 256
REP = HQ // HKV  # 2
NCORES = 8
SCALE = HD ** (-0.5)


def build_bass():
    import concourse.bass as bass  # noqa: F401
    import concourse.mybir as mybir
    import concourse.tile as tile
    from concourse import bacc
    from contextlib import ExitStack

    f32 = mybir.dt.float32
    bf16 = mybir.dt.bfloat16
    Alu = mybir.AluOpType
    Act = mybir.ActivationFunctionType

    nc = bacc.Bacc("TRN2", target_bir_lowering=False, debug=False,
                   num_devices=NCORES)

    # DRAM inputs (host-prepped layouts; see _prep_inputs)
    xT = nc.dram_tensor("xT", [128, 24 * B], bf16, kind="ExternalInput").ap()
    wq = nc.dram_tensor("wq", [3, 128, 8192], bf16, kind="ExternalInput").ap()
    kt = nc.dram_tensor("kt", [8, 128, 8192], bf16, kind="ExternalInput").ap()
    vt = nc.dram_tensor("vt", [8, 128, 8192], bf16, kind="ExternalInput").ap()
    wo = nc.dram_tensor("wo", [128, 4 * DIM], bf16, kind="ExternalInput").ap()
    fm = nc.dram_tensor("fm", [16, C], f32, kind="ExternalInput").ap()
    cs4 = nc.dram_tensor("cs4", [128, 4], f32, kind="ExternalInput").ap()
    identf = nc.dram_tensor("identf", [128, 128], f32,
                            kind="ExternalInput").ap()
    cmask = nc.dram_tensor("cmask", [128, 128], f32, kind="ExternalInput").ap()
    dup = nc.dram_tensor("dup", [B, 16], f32, kind="ExternalInput").ap()
    ones1 = nc.dram_tensor("ones1", [1, 128], f32, kind="ExternalInput").ap()
    mkv = nc.dram_tensor("mkv", [16, 1], f32, kind="ExternalInput").ap()
    y = nc.dram_tensor("y", [B, DIM], f32, kind="ExternalOutput").ap()

    with tile.TileContext(nc) as tc, ExitStack() as stk:
        io = stk.enter_context(tc.tile_pool(name="io", bufs=1))
        # one shared ring for all big streaming loads (W_qkv, K, V):
        # deep enough that V prefetch runs ahead while softmax/probsT
        # compute, keeping the DMA queue always busy.
        st = stk.enter_context(tc.tile_pool(name="st", bufs=7))
        ap_ = stk.enter_context(tc.tile_pool(name="ap", bufs=2))
        ps = stk.enter_context(tc.tile_pool(name="ps", bufs=8, space="PSUM"))

        # ---- small persistent constants ----
        xT_sb = io.tile([128, 24 * B], bf16, tag="xT")
        nc.sync.dma_start(xT_sb[:], xT)
        cs_sb = io.tile([128, 4], f32, tag="cs")
        nc.sync.dma_start(cs_sb[:], cs4)
        id_sb = io.tile([128, 128], f32, tag="id")
        nc.sync.dma_start(id_sb[:], identf)
        cm_sb = io.tile([128, 128], f32, tag="cm")
        nc.sync.dma_start(cm_sb[:], cmask)
        dup_sb = io.tile([B, 16], f32, tag="dup")
        nc.sync.dma_start(dup_sb[:], dup)
        on_sb = io.tile([1, 128], f32, tag="on")
        nc.sync.dma_start(on_sb[:], ones1)
        mkv_sb = io.tile([16, 1], f32, tag="mkv")
        nc.sync.dma_start(mkv_sb[:], mkv)
        fm_sb = io.tile([16, C], f32, tag="fm")
        nc.sync.dma_start(fm_sb[:], fm)
        cos_s, sin_s = cs_sb[:, 0:1], cs_sb[:, 1:2]
        cos_p, sin_p = cs_sb[:, 2:3], cs_sb[:, 3:4]

        # ---- phase 1: qkv rows = x @ Wq_shard; x stationary, W moving ----
        psq = [ps.tile([B, 512], f32, tag="ps", name=f"psq{j}")
               for j in range(2)]
        for ci in range(3):
            wt = st.tile([128, 8192], bf16, tag="st", name="wt")
            nc.sync.dma_start(wt[:], wq[ci])
            for il in range(8):
                t = ci * 8 + il
                lhsT = xT_sb[:, t * B:(t + 1) * B]
                for j2 in range(2):
                    nc.tensor.matmul(psq[j2][:], lhsT,
                                     wt[:, il * 1024 + j2 * 512:
                                        il * 1024 + (j2 + 1) * 512],
                                     start=(t == 0), stop=(t == 23))
        qkv_sb = io.tile([B, 1024], f32, tag="qkv")
        nc.scalar.copy(qkv_sb[:, 0:512], psq[0][:])
        nc.scalar.copy(qkv_sb[:, 512:1024], psq[1][:])
        # v_new rows, straight to bf16
        vn_sb = io.tile([B, 256], bf16, tag="vn")
        nc.scalar.copy(vn_sb[:], psq[1][:, 256:512])

        # ---- phase 2: transposes + rope + batch-masked q tiles ----
        # q slices [8, 128] -> [128, 8] per (h, dh); k slices likewise
        qt_raw = [[io.tile([128, B], f32, tag=f"qr{h}{dh}")
                   for dh in range(2)] for h in range(2)]
        for h in range(2):
            for dh in range(2):
                pt = ps.tile([128, B], f32, tag="ps")
                nc.tensor.transpose(
                    pt[:], qkv_sb[:, h * 256 + dh * 128:
                                  h * 256 + (dh + 1) * 128],
                    id_sb[:B, :B])
                nc.scalar.copy(qt_raw[h][dh][:], pt[:])
        kn_raw = [io.tile([128, B], f32, tag=f"kr{dh}") for dh in range(2)]
        for dh in range(2):
            pt = ps.tile([128, B], f32, tag="ps")
            nc.tensor.transpose(pt[:], qkv_sb[:, 512 + dh * 128:
                                              512 + (dh + 1) * 128],
                                id_sb[:B, :B])
            nc.scalar.copy(kn_raw[dh][:], pt[:])

        def rope(c1, c2, cosa, sina, out1, out2):
            ta = io.tile([128, B], f32, tag="rta", name="rta")
            tb = io.tile([128, B], f32, tag="rtb", name="rtb")
            nc.vector.tensor_scalar_mul(ta[:], c1, cosa)
            nc.vector.tensor_scalar_mul(tb[:], c2, sina)
            nc.vector.tensor_tensor(out1, ta[:], tb[:], op=Alu.subtract)
            nc.vector.tensor_scalar_mul(ta[:], c1, sina)
            nc.vector.tensor_scalar_mul(tb[:], c2, cosa)
            nc.vector.tensor_tensor(out2, ta[:], tb[:], op=Alu.add)

        # qTh[dh] [128, 16] f32, col = 2b + h
        qTh = [io.tile([128, 16], f32, tag=f"qTh{dh}") for dh in range(2)]
        for h in range(2):
            o1 = qTh[0][:].rearrange("p (b r) -> p r b", r=2)[:, h]
            o2 = qTh[1][:].rearrange("p (b r) -> p r b", r=2)[:, h]
            rope(qt_raw[h][0][:], qt_raw[h][1][:], cos_s, sin_s, o1, o2)
        # knT[dh] [128, 8] bf16
        knT = [io.tile([128, B], bf16, tag=f"knT{dh}") for dh in range(2)]
        rope(kn_raw[0][:], kn_raw[1][:], cos_p, sin_p, knT[0][:], knT[1][:])

        # batch-masked q tiles (bf16): only cols 2b, 2b+1 nonzero
        Mt = [[io.tile([128, 16], bf16, tag=f"Mt{b}_{dh}")
               for dh in range(2)] for b in range(B)]
        for b in range(B):
            for dh in range(2):
                nc.vector.tensor_tensor(Mt[b][dh][:], qTh[dh][:],
                                        cm_sb[:, b * 16:(b + 1) * 16],
                                        op=Alu.mult)

        # ---- s_new[16,1] (+ mask[kv]) ----
        psn = ps.tile([16, 1], f32, tag="ps")
        for b in range(B):
            for dh in range(2):
                nc.tensor.matmul(psn[:], Mt[b][dh][:], knT[dh][:, b:b + 1],
                                 start=(b == 0 and dh == 0),
                                 stop=(b == B - 1 and dh == 1))
        s_new = io.tile([16, 1], f32, tag="snew")
        nc.vector.tensor_scalar_add(s_new[:], psn[:], mkv_sb[:, 0:1])

        # ---- phase 3: scores [16, C] via masked accumulation ----
        scores = io.tile([16, C], f32, tag="scores")
        mparts = io.tile([16, 8], f32, tag="mparts")
        for g in range(8):
            ktile = st.tile([128, 8192], bf16, tag="st", name="ktile")
            nc.sync.dma_start(ktile[:], kt[g])
            pch = ps.tile([16, 512], f32, tag="ps")
            for b in range(B):
                for dh in range(2):
                    nc.tensor.matmul(pch[:], Mt[b][dh][:],
                                     ktile[:, (b * 2 + dh) * 512:
                                           (b * 2 + dh + 1) * 512],
                                     start=(b == 0 and dh == 0),
                                     stop=(b == B - 1 and dh == 1))
            ssl = slice(g * 512, (g + 1) * 512)
            nc.vector.tensor_tensor(scores[:, ssl], pch[:], fm_sb[:, ssl],
                                    op=Alu.add)
            nc.vector.tensor_reduce(mparts[:, g:g + 1], scores[:, ssl],
                                    axis=mybir.AxisListType.X, op=Alu.max)

        # out-proj weights: fetch between K and V streams
        wo_sb = io.tile([128, 4 * DIM], bf16, tag="wo")
        nc.sync.dma_start(wo_sb[:], wo)

        # ---- phase 4: softmax (new token via rank-1 fixup) ----
        m1 = io.tile([16, 1], f32, tag="m1")
        nc.vector.tensor_reduce(m1[:], mparts[:], axis=mybir.AxisListType.X,
                                op=Alu.max)
        tmax = io.tile([16, 1], f32, tag="tmax")
        nc.vector.tensor_tensor(tmax[:], m1[:], s_new[:], op=Alu.max)
        negmax = io.tile([16, 1], f32, tag="negmax")
        nc.vector.tensor_scalar_mul(negmax[:], tmax[:], -1.0)
        sumz = io.tile([16, 1], f32, tag="sumz")
        nc.scalar.activation(scores[:], scores[:], Act.Exp, bias=negmax[:],
                             accum_out=sumz[:])
        p_kv = io.tile([16, 1], f32, tag="pkv")
        nc.scalar.activation(p_kv[:], s_new[:], Act.Exp, bias=negmax[:])
        norm = io.tile([16, 1], f32, tag="norm")
        nc.vector.tensor_tensor(norm[:], sumz[:], p_kv[:], op=Alu.add)
        rnorm = io.tile([16, 1], f32, tag="rnorm")
        nc.vector.reciprocal(rnorm[:], norm[:])
        # rnB [128, 16]: rnorm broadcast down partitions
        prt = ps.tile([1, 16], f32, tag="ps")
        nc.tensor.transpose(prt[:], rnorm[:], id_sb[:16, :16])
        rnT = io.tile([1, 16], f32, tag="rnT")
        nc.scalar.copy(rnT[:], prt[:])
        prb = ps.tile([128, 16], f32, tag="ps")
        nc.tensor.matmul(prb[:], on_sb[:], rnT[:], start=True, stop=True)
        rnB = io.tile([128, 16], f32, tag="rnB")
        nc.scalar.copy(rnB[:], prb[:])
        # selPn[b', 2b+r] = delta(b',b) * p_new[2b+r] * rnorm[2b+r]  (bf16)
        pnt = ps.tile([1, 16], f32, tag="ps")
        nc.tensor.transpose(pnt[:], p_kv[:], id_sb[:16, :16])
        pkvnT = io.tile([1, 16], f32, tag="pkvnT")
        nc.scalar.copy(pkvnT[:], pnt[:])
        pob = ps.tile([B, 16], f32, tag="ps")
        nc.tensor.matmul(pob[:], on_sb[:, 0:B], pkvnT[:], start=True,
                         stop=True)
        pkvB = io.tile([B, 16], f32, tag="pkvB")
        nc.scalar.copy(pkvB[:], pob[:])
        selP = io.tile([B, 16], f32, tag="selP")
        nc.vector.tensor_tensor(selP[:], dup_sb[:], pkvB[:], op=Alu.mult)
        selPn = io.tile([B, 16], bf16, tag="selPn")
        nc.vector.tensor_tensor(selPn[:], selP[:], rnB[:B, :], op=Alu.mult)

        # ---- phase 5: probsT (normalized, bf16) via PE transpose ----
        probsT = io.tile([128, 32 * 16], bf16, tag="probsT")
        for ct in range(32):
            pt = ps.tile([128, 16], f32, tag="ps")
            nc.tensor.transpose(pt[:], scores[:, ct * 128:(ct + 1) * 128],
                                id_sb[:16, :16])
            nc.vector.tensor_tensor(probsT[:, ct * 16:(ct + 1) * 16], pt[:],
                                    rnB[:], op=Alu.mult)

        # ---- phase 6: attn = probs @ V per batch (M=2), transpose to aT ----
        aT4 = [io.tile([128, B], bf16, tag=f"aT{t}") for t in range(4)]
        for b in range(B):
            vtile = st.tile([128, 8192], bf16, tag="st", name="vtile")
            nc.sync.dma_start(vtile[:], vt[b])
            pab = ps.tile([2, 256], f32, tag="ps")
            for ct in range(32):
                nc.tensor.matmul(pab[:],
                                 probsT[:, ct * 16 + 2 * b:
                                        ct * 16 + 2 * b + 2],
                                 vtile[:, ct * 256:(ct + 1) * 256],
                                 start=(ct == 0), stop=False)
            nc.tensor.matmul(pab[:], selPn[:, 2 * b:2 * b + 2], vn_sb[:],
                             start=False, stop=True)
            attn_b = ap_.tile([2, 256], f32, tag="attn")
            nc.scalar.copy(attn_b[:], pab[:])
            for dh in range(2):
                pta = ps.tile([128, 2], f32, tag="ps")
                nc.tensor.transpose(pta[:],
                                    attn_b[:, dh * 128:(dh + 1) * 128],
                                    id_sb[:2, :2])
                for h in range(2):
                    nc.scalar.copy(aT4[h * 2 + dh][:, b:b + 1],
                                   pta[:, h:h + 1])

        # ---- phase 7: y = attn @ Wo_shard ----
        y_sb = io.tile([B, DIM], f32, tag="ysb")
        for n in range(6):
            py = ps.tile([B, 512], f32, tag="ps")
            for t in range(4):
                nc.tensor.matmul(py[:], aT4[t][:],
                                 wo_sb[:, t * DIM + n * 512:
                                       t * DIM + (n + 1) * 512],
                                 start=(t == 0), stop=(t == 3))
            nc.scalar.copy(y_sb[:, n * 512:(n + 1) * 512], py[:])
        nc.sync.dma_start(y, y_sb[:])

    nc.compile()
    return nc


_CACHED = {}


def _get_bass():
    if "nc" not in _CACHED:
        _CACHED["nc"] = build_bass()
    return _CACHED["nc"]


def _prep_inputs(x, freqs_cos, freqs_sin, kv, k_cache, v_cache, mask,
                 W_qkv, W_out):
    import ml_dtypes

    bf = ml_dtypes.bfloat16
    x2 = np.asarray(x, np.float32).reshape(B, DIM)
    xT192 = np.ascontiguousarray(
        x2.T.reshape(24, 128, B).transpose(1, 0, 2).reshape(128, 24 * B)
    ).astype(bf)
    cos = np.asarray(freqs_cos, np.float32)[0]
    sin = np.asarray(freqs_sin, np.float32)[0]
    cs4 = np.ascontiguousarray(
        np.stack([cos * SCALE, sin * SCALE, cos, sin], 1), np.float32)
    kvp = int(np.asarray(kv).reshape(-1)[0])
    maskr = np.asarray(mask, np.float32)
    fm = np.tile(maskr, (16, 1)).astype(np.float32)
    fm[:, kvp] -= 1e30
    mkv = np.full((16, 1), maskr[0, kvp], np.float32)
    identf = np.eye(128, dtype=np.float32)
    dupm = np.zeros((B, 16), np.float32)
    for b in range(B):
        dupm[b, 2 * b] = 1.0
        dupm[b, 2 * b + 1] = 1.0
    cmask = np.zeros((128, 128), np.float32)
    for b in range(B):
        cmask[:, b * 16 + 2 * b] = 1.0
        cmask[:, b * 16 + 2 * b + 1] = 1.0
    ones1 = np.ones((1, 128), np.float32)
    kc = np.asarray(k_cache, np.float32)
    vc = np.asarray(v_cache, np.float32)
    Wq = np.asarray(W_qkv, np.float32)
    Wo = np.asarray(W_out, np.float32)

    in_maps = []
    for m in range(NCORES):
        wq_shard = np.concatenate([
            Wq[:, 2 * m * HD:(2 * m + 2) * HD],
            Wq[:, HQ * HD + m * HD: HQ * HD + (m + 1) * HD],
            Wq[:, (HQ + HKV) * HD + m * HD: (HQ + HKV) * HD + (m + 1) * HD],
        ], axis=1)  # [3072, 1024]
        wq3 = np.ascontiguousarray(
            wq_shard.reshape(3, 8, 128, 1024).transpose(0, 2, 1, 3)
            .reshape(3, 128, 8192)).astype(bf)
        kc_m = kc[:, :, m, :]  # [B, C, 256]
        kt8 = np.ascontiguousarray(
            kc_m.reshape(B, 8, 512, 2, 128).transpose(1, 4, 0, 3, 2)
            .reshape(8, 128, 8192)).astype(bf)
        vc_m = vc[:, :, m, :]  # [B, C, 256]
        vt8 = np.ascontiguousarray(
            vc_m.reshape(B, 32, 128, 256).transpose(0, 2, 1, 3)
            .reshape(B, 128, 8192)).astype(bf)
        wo_shard = np.ascontiguousarray(
            Wo[m * 2 * HD:(m + 1) * 2 * HD, :].reshape(4, 128, DIM)
            .transpose(1, 0, 2).reshape(128, 4 * DIM)).astype(bf)
        in_maps.append({
            "xT": xT192, "wq": wq3, "kt": kt8, "vt": vt8, "wo": wo_shard,
            "fm": fm, "cs4": cs4, "identf": identf, "cmask": cmask,
            "dup": dupm, "ones1": ones1, "mkv": mkv,
        })
    return in_maps


def _run(inputs, trace=False):
    from concourse.bass_utils import run_bass_kernel_spmd
    nc = _get_bass()
    in_maps = _prep_inputs(**inputs)
    res = run_bass_kernel_spmd(nc, in_maps, core_ids=list(range(NCORES)),
                               trace=trace)
    parts = [r["y"] for r in res.results]
    out = np.sum(np.stack(parts, 0), 0, dtype=np.float32)
    return out.reshape(B, S, DIM), res


def kernel(**inputs):
    out, _ = _run(inputs, trace=False)
    return out
